# revision 17
# baseline (speedup 1.0000x reference)
"""Multi-head attention (B=4,S=2048,E=1024,H=16,Dh=64) on 8 TRN2 NeuronCores.

Sharding: core c handles batch b=c//2 and heads hh*8..hh*8+8 (hh=c%2).
Returns (out [B,S,E], attn_weights [B,H,S,S]) matching the reference.

Per-core device program (all matmuls float32r, fp32 I/O):
  1. Projections: qhT/khT [d_local, S] (head-dim on partitions), vh [S, d_local]
     natural layout augmented with a ones column per head (row-sum trick).
  2. T-phase per (head, i-tile of 512): scores.T blocks [128 j, 512 i] -> exp
     (scale=1/8, bias=-C) -> A@V accumulation; vh's ones column makes PSUM
     row 64 the softmax denominators. Denominators -> ln -> per-partition
     bias tiles (PE row-transpose); reciprocal row -> PE-broadcast -> A@V
     normalization at evacuation.
  3. U-phase per (head, i-tile of 128): scores blocks [128 i, 512 j] -> exp
     with bias = -C - ln(sum_i) => normalized attention weights directly;
     causal-masked entries are exact zeros; upper triangle blocks are never
     written (outputs are zero-initialized on device).
  4. Out-projection partial (this core's 512 merged dims): out_part [S, E];
     host sums the two per-batch partials and adds bo.
"""
import sys
import numpy as np
from contextlib import ExitStack

sys.path.insert(0, "/opt/trn_rl_repo")

import jax

# Persist compiled executables (incl. the embedded NEFF) across processes so
# repeat invocations skip the multi-minute neuronxcc compile.
try:
    jax.config.update("jax_compilation_cache_dir", "/tmp/jax_mha_cache")
    jax.config.update("jax_persistent_cache_min_compile_time_secs", 0.0)
    jax.config.update("jax_persistent_cache_min_entry_size_bytes", 0)
except Exception:
    pass

import concourse.bass as bass
import concourse.mybir as mybir
import concourse.tile as tile
from concourse import bacc
from concourse.bass_utils import run_bass_kernel_spmd

F32 = mybir.dt.float32
F32R = mybir.dt.float32r
AF = mybir.ActivationFunctionType
OP = mybir.AluOpType

B, S, E, H, Dh = 4, 2048, 1024, 16, 64
HPC = H // 2          # 8 heads per core
JW = HPC * Dh         # 512 local merged width
NI = S // 128         # 16 i-tiles (128-wide)
NI5 = S // 512        # 4 i-tiles (512-wide)
NJ = S // 128         # 16 j-tiles (128-wide, T-phase)
NJ5 = S // 512        # 4 j-tiles (512-wide, U-phase)
NE = E // 128         # 8 e-tiles
SCALE = 1.0 / np.sqrt(Dh)
C_OFF = 12.0          # exp offset; softmax is invariant to it

_programs = {}


def build_program(causal: bool):
    nc = bacc.Bacc("TRN2", target_bir_lowering=False, debug=False, num_devices=8)

    # ---- DRAM I/O ----
    xqT_d = nc.dram_tensor("xqT", [E, S], F32R, kind="ExternalInput").ap()
    xkT_d = nc.dram_tensor("xkT", [E, S], F32R, kind="ExternalInput").ap()
    xvT_d = nc.dram_tensor("xvT", [E, S], F32R, kind="ExternalInput").ap()
    wqT_d = nc.dram_tensor("wqT", [E, JW], F32R, kind="ExternalInput").ap()
    wkT_d = nc.dram_tensor("wkT", [E, JW], F32R, kind="ExternalInput").ap()
    wvT_d = nc.dram_tensor("wvT", [E, JW], F32R, kind="ExternalInput").ap()
    bqp_d = nc.dram_tensor("bqp", [128, 4], F32, kind="ExternalInput").ap()
    bkp_d = nc.dram_tensor("bkp", [128, 4], F32, kind="ExternalInput").ap()
    bvr_d = nc.dram_tensor("bvr", [1, JW], F32R, kind="ExternalInput").ap()
    woT_d = nc.dram_tensor("woT", [JW, E], F32R, kind="ExternalInput").ap()
    mU_d = nc.dram_tensor("maskU", [128, 4, 512], F32, kind="ExternalInput").ap()
    mT_d = nc.dram_tensor("maskT", [128, 4, 512], F32, kind="ExternalInput").ap()

    attn_d = nc.dram_tensor("attn", [HPC, S, S], F32, kind="ExternalOutput").ap()
    outp_d = nc.dram_tensor("outp", [S, E], F32, kind="ExternalOutput").ap()

    with tile.TileContext(nc) as tc, ExitStack() as ctx:
        perm = ctx.enter_context(tc.tile_pool(name="perm", bufs=1))

        # persistent tensors
        qhT = perm.tile([128, 4, S], F32R)      # partitions: local head dims
        khT = perm.tile([128, 4, S], F32R)
        vh = perm.tile([128, NJ, HPC * (Dh + 1)], F32R)  # [s-chunk, 8*(64+1)]
        merged = perm.tile([128, 4, S], F32R)   # normalized (A@V).T
        if causal:
            mU = perm.tile([128, 4, 512], F32)
            mT = perm.tile([128, 4, 512], F32)
            nc.sync.dma_start(mU[:], mU_d[:])
            nc.sync.dma_start(mT[:], mT_d[:])
        bqp = perm.tile([128, 4], F32)
        bkp = perm.tile([128, 4], F32)
        nc.sync.dma_start(bqp[:], bqp_d[:])
        nc.sync.dma_start(bkp[:], bkp_d[:])
        bvr = perm.tile([1, JW], F32R)
        nc.sync.dma_start(bvr[:], bvr_d[:])
        ones_r = perm.tile([1, 128], F32R)
        nc.vector.memset(ones_r[:].bitcast(F32), 1.0)
        ones1 = perm.tile([1, 1], F32)
        nc.vector.memset(ones1[:], 1.0)
        biasc = perm.tile([128, 1], F32)
        nc.vector.memset(biasc[:], -C_OFF)
        # ones columns of vh (col 64 of each head's 65-wide group)
        vh_g = vh[:].rearrange("p nj (h d) -> p nj h d", d=Dh + 1)
        nc.vector.memset(vh_g[:, :, :, Dh:Dh + 1].bitcast(F32), 1.0)

        # ---------------- projections ----------------
        with tc.tile_pool(name="wproj", bufs=1) as wpool, \
             tc.tile_pool(name="xstage", bufs=3) as xpool, \
             tc.tile_pool(name="pproj", bufs=1, space="PSUM") as ppool:
            wq = wpool.tile([128, NE, JW], F32R)
            wk = wpool.tile([128, NE, JW], F32R)
            wv = wpool.tile([128, NE, JW], F32R)
            nc.sync.dma_start(wq[:], wqT_d.rearrange("(ne p) j -> p ne j", p=128))
            nc.sync.dma_start(wk[:], wkT_d.rearrange("(ne p) j -> p ne j", p=128))
            nc.sync.dma_start(wv[:], wvT_d.rearrange("(ne p) j -> p ne j", p=128))

            for st in range(4):            # s-tiles of 512
                s0 = st * 512
                # --- q and k: out [j, s] ---
                for which, xd, w, dst, bias in (
                    ("q", xqT_d, wq, qhT, bqp), ("k", xkT_d, wk, khT, bkp),
                ):
                    pss = [ppool.tile([128, 512], F32, tag=f"pqk{j}", name=f"pqk{j}")
                           for j in range(4)]
                    for e in range(NE):
                        xs = xpool.tile([128, 512], F32R, tag="xs")
                        nc.sync.dma_start(xs[:], xd[e * 128:(e + 1) * 128, s0:s0 + 512])
                        for j in range(4):
                            nc.tensor.matmul(
                                pss[j][:], w[:, e, j * 128:(j + 1) * 128], xs[:],
                                start=(e == 0), stop=(e == NE - 1))
                    for j in range(4):
                        nc.scalar.activation(dst[:, j, s0:s0 + 512], pss[j][:],
                                             AF.Identity, bias=bias[:, j:j + 1])
                # --- v: out [s, d] with ones-row bias augmentation ---
                pss = [ppool.tile([128, JW], F32, tag=f"pv{sc}", name=f"pv{sc}") for sc in range(4)]
                for e in range(NE):
                    xs = xpool.tile([128, 512], F32R, tag="xs")
                    nc.sync.dma_start(xs[:], xvT_d[e * 128:(e + 1) * 128, s0:s0 + 512])
                    for sc in range(4):
                        nc.tensor.matmul(
                            pss[sc][:], xs[:, sc * 128:(sc + 1) * 128], wv[:, e, :],
                            start=(e == 0), stop=False)
                for sc in range(4):
                    nc.tensor.matmul(pss[sc][:], ones_r[:], bvr[:],
                                     start=False, stop=True)
                    gsc = st * 4 + sc
                    nc.vector.tensor_copy(
                        vh[:, gsc, :].rearrange("p (h d) -> p h d", d=Dh + 1)[:, :, 0:Dh],
                        pss[sc][:].rearrange("p (h d) -> p h d", d=Dh))

        # ---------------- attention ----------------
        with tc.tile_pool(name="pst", bufs=2, space="PSUM") as pst, \
             tc.tile_pool(name="pav", bufs=2, space="PSUM") as pav, \
             tc.tile_pool(name="prb", bufs=1, space="PSUM") as prb, \
             tc.tile_pool(name="ppb", bufs=1, space="PSUM") as ppb, \
             tc.tile_pool(name="psu", bufs=2, space="PSUM") as psu, \
             tc.tile_pool(name="expt", bufs=4) as expt_pool, \
             tc.tile_pool(name="rows", bufs=2) as rows_pool, \
             tc.tile_pool(name="bias", bufs=2) as bias_pool, \
             tc.tile_pool(name="arow", bufs=3) as arow_pool:
            for h in range(HPC):
                base = (h % 2) * 64
                jt = h // 2
                bias_all = bias_pool.tile([128, NI], F32, tag="bias_all")
                # ---- T phase: scores.T, exp, A@V, denominators ----
                for ti5 in range(NI5):
                    i0 = ti5 * 512
                    ntj = 4 * (ti5 + 1) if causal else NJ
                    av = pav.tile([Dh + 1, 512], F32, tag="av")
                    for tj in range(ntj):
                        ps = pst.tile([128, 512], F32, tag="pst")
                        nc.tensor.matmul(ps[:], khT[base:base + 64, jt, tj * 128:(tj + 1) * 128],
                                         qhT[base:base + 64, jt, i0:i0 + 512],
                                         start=True, stop=True)
                        if causal and tj >= 4 * ti5:
                            nc.vector.tensor_tensor(out=ps[:], in0=ps[:],
                                                    in1=mT[:, tj - 4 * ti5, :], op=OP.add)
                        ex = expt_pool.tile([128, 512], F32R, tag="expt")
                        nc.scalar.activation(ex[:], ps[:], AF.Exp,
                                             bias=biasc[:], scale=float(SCALE))
                        nc.tensor.matmul(av[:], vh[:, tj, h * (Dh + 1):(h + 1) * (Dh + 1)],
                                         ex[:], start=(tj == 0), stop=(tj == ntj - 1))
                    # denominators: av row 64
                    sums = rows_pool.tile([1, 512], F32, tag="sums")
                    nc.vector.tensor_copy(sums[:], av[Dh:Dh + 1, :])
                    ln_r = rows_pool.tile([1, 512], F32, tag="ln")
                    nc.scalar.activation(ln_r[:], sums[:], AF.Ln, bias=0.0, scale=1.0)
                    ngc = rows_pool.tile([1, 512], F32, tag="ngc")
                    nc.vector.tensor_scalar(out=ngc[:], in0=ln_r[:], scalar1=-1.0,
                                            scalar2=-C_OFF, op0=OP.mult, op1=OP.add)
                    r_row = rows_pool.tile([1, 512], F32R, tag="rr")
                    nc.scalar.activation(r_row[:], ln_r[:], AF.Exp, bias=0.0, scale=-1.0)
                    # bias tiles: transpose ngc [1,512] -> [128,1] x4
                    pb = ppb.tile([128, 4], F32, tag="pb")
                    for k in range(4):
                        nc.tensor.transpose(pb[:, k:k + 1], ngc[:, k * 128:(k + 1) * 128],
                                            ones1[:])
                    nc.vector.tensor_copy(bias_all[:, ti5 * 4:(ti5 + 1) * 4], pb[:])
                    # broadcast 1/sums and normalize A@V into merged
                    rb = prb.tile([Dh, 512], F32, tag="rb")
                    nc.tensor.matmul(rb[:], ones_r[:, 0:Dh], r_row[:],
                                     start=True, stop=True)
                    rb_sb = rows_pool.tile([Dh, 512], F32, tag="rbsb")
                    nc.vector.tensor_copy(rb_sb[:], rb[:])
                    nc.vector.tensor_tensor(
                        out=merged[base:base + 64, jt, i0:i0 + 512],
                        in0=av[0:Dh, :], in1=rb_sb[:], op=OP.mult)
                # ---- U phase: normalized attention weights out ----
                for ti in range(NI):
                    ntj5 = (ti // 4) + 1 if causal else NJ5
                    ar = arow_pool.tile([128, S], F32, tag="arow")
                    for tj5 in range(ntj5):
                        j0 = tj5 * 512
                        ps = psu.tile([128, 512], F32, tag="psu")
                        nc.tensor.matmul(ps[:], qhT[base:base + 64, jt, ti * 128:(ti + 1) * 128],
                                         khT[base:base + 64, jt, j0:j0 + 512],
                                         start=True, stop=True)
                        if causal and tj5 == ntj5 - 1:
                            nc.vector.tensor_tensor(out=ps[:], in0=ps[:],
                                                    in1=mU[:, ti % 4, :], op=OP.add)
                        nc.scalar.activation(ar[:, j0:j0 + 512], ps[:], AF.Exp,
                                             bias=bias_all[:, ti:ti + 1], scale=float(SCALE))
                    nc.sync.dma_start(attn_d[h, ti * 128:(ti + 1) * 128, 0:ntj5 * 512],
                                      ar[:, 0:ntj5 * 512])

        # ---------------- out projection (partial) ----------------
        with tc.tile_pool(name="wo", bufs=1) as wopool, \
             tc.tile_pool(name="po", bufs=3, space="PSUM") as popool, \
             tc.tile_pool(name="oev", bufs=3) as oevpool:
            wo = wopool.tile([128, 4, E], F32R)
            nc.sync.dma_start(wo[:], woT_d.rearrange("(dt p) e -> p dt e", p=128))
            for sc in range(NJ):           # 16 s-chunks of 128
                for et in range(2):        # e-tiles of 512
                    ps = popool.tile([128, 512], F32, tag="po")
                    for dt in range(4):
                        nc.tensor.matmul(ps[:], merged[:, dt, sc * 128:(sc + 1) * 128],
                                         wo[:, dt, et * 512:(et + 1) * 512],
                                         start=(dt == 0), stop=(dt == 3))
                    ot = oevpool.tile([128, 512], F32, tag="oev")
                    nc.vector.tensor_copy(ot[:], ps[:])
                    nc.sync.dma_start(
                        outp_d[sc * 128:(sc + 1) * 128, et * 512:(et + 1) * 512], ot[:])

    nc.compile()
    return nc


def _get_program(causal: bool):
    if causal not in _programs:
        _programs[causal] = build_program(causal)
    return _programs[causal]


def _host_masks():
    r = np.arange(128)[:, None]
    c = np.arange(512)[None, :]
    mU = np.zeros((4, 128, 512), dtype=np.float32)
    mT = np.zeros((4, 128, 512), dtype=np.float32)
    for p in range(4):
        mU[p] = np.where(c <= 128 * p + r, 0.0, -1e9)
        mT[p] = np.where(c >= r + 128 * p, 0.0, -1e9)
    # device layout: [128 partitions, 4 patterns, 512]
    return (np.ascontiguousarray(mU.transpose(1, 0, 2)),
            np.ascontiguousarray(mT.transpose(1, 0, 2)))


def _numpy_fallback(q, k, v, mask, Wq, bq, Wk, bk, Wv, bv, Wo, bo):
    def split_heads(x):
        return x.reshape(B, S, H, Dh).transpose(0, 2, 1, 3)
    qh = split_heads(q @ Wq.T + bq)
    kh = split_heads(k @ Wk.T + bk)
    vh = split_heads(v @ Wv.T + bv)
    scores = np.einsum("bhqd,bhkd->bhqk", qh, kh) * np.float32(SCALE)
    scores = np.where(np.asarray(mask) == 0, np.float32(-1e9), scores)
    m = scores.max(axis=-1, keepdims=True)
    e = np.exp(scores - m)
    aw = e / e.sum(axis=-1, keepdims=True)
    attn_out = np.einsum("bhqk,bhkd->bhqd", aw, vh)
    mg = attn_out.transpose(0, 2, 1, 3).reshape(B, S, E)
    return (mg @ Wo.T + bo).astype(np.float32), aw.astype(np.float32)


def kernel(q, k, v, mask, Wq, bq, Wk, bk, Wv, bv, Wo, bo):
    q = np.asarray(q, dtype=np.float32)
    k = np.asarray(k, dtype=np.float32)
    v = np.asarray(v, dtype=np.float32)
    mask = np.asarray(mask)
    Wq, bq = np.asarray(Wq, np.float32), np.asarray(bq, np.float32)
    Wk, bk = np.asarray(Wk, np.float32), np.asarray(bk, np.float32)
    Wv, bv = np.asarray(Wv, np.float32), np.asarray(bv, np.float32)
    Wo, bo = np.asarray(Wo, np.float32), np.asarray(bo, np.float32)

    m2 = np.broadcast_to(mask, (1, 1, S, S)).reshape(S, S)
    tril = np.tril(np.ones((S, S), dtype=m2.dtype))
    if np.array_equal(m2, tril):
        causal = True
    elif np.all(m2 != 0):
        causal = False
    else:
        return _numpy_fallback(q, k, v, mask, Wq, bq, Wk, bk, Wv, bv, Wo, bo)

    nc = _get_program(causal)
    mU, mT = _host_masks()

    in_maps = []
    for c in range(8):
        b, hh = c // 2, c % 2
        sl = slice(hh * JW, (hh + 1) * JW)
        in_maps.append({
            "xqT": np.ascontiguousarray(q[b].T),
            "xkT": np.ascontiguousarray(k[b].T),
            "xvT": np.ascontiguousarray(v[b].T),
            "wqT": np.ascontiguousarray(Wq[sl, :].T),
            "wkT": np.ascontiguousarray(Wk[sl, :].T),
            "wvT": np.ascontiguousarray(Wv[sl, :].T),
            "bqp": np.ascontiguousarray(bq[sl].reshape(4, 128).T),
            "bkp": np.ascontiguousarray(bk[sl].reshape(4, 128).T),
            "bvr": bv[sl].reshape(1, JW).copy(),
            "woT": np.ascontiguousarray(Wo[:, sl].T),
            "maskU": mU, "maskT": mT,
        })

    res = run_bass_kernel_spmd(nc, in_maps, list(range(8)))

    aw = np.empty((B, H, S, S), dtype=np.float32)
    out = np.empty((B, S, E), dtype=np.float32)
    for b in range(B):
        aw[b, 0:HPC] = res.results[2 * b]["attn"]
        aw[b, HPC:H] = res.results[2 * b + 1]["attn"]
        out[b] = res.results[2 * b]["outp"] + res.results[2 * b + 1]["outp"] + bo
    return out, aw


# revision 20
# speedup vs baseline: 1.3348x; 1.3348x over previous
"""Multi-head attention (B=4,S=2048,E=1024,H=16,Dh=64) on 8 TRN2 NeuronCores.

Sharding: core c handles batch b=c//2 and heads hh*8..hh*8+8 (hh=c%2).
Returns (out [B,S,E], attn_weights [B,H,S,S]) matching the reference.

Per-core device program (all matmuls float32r, fp32 I/O):
  1. Projections: qhT/khT [d_local, S] (head-dim on partitions), vh [S, d_local]
     natural layout augmented with a ones column per head (row-sum trick).
  2. T-phase per (head, i-tile of 512): scores.T blocks [128 j, 512 i] -> exp
     (scale=1/8, bias=-C) -> A@V accumulation; vh's ones column makes PSUM
     row 64 the softmax denominators. Denominators -> ln -> per-partition
     bias tiles (PE row-transpose); reciprocal row -> PE-broadcast -> A@V
     normalization at evacuation.
  3. U-phase per (head, i-tile of 128): scores blocks [128 i, 512 j] -> exp
     with bias = -C - ln(sum_i) => normalized attention weights directly;
     causal-masked entries are exact zeros; upper triangle blocks are never
     written (outputs are zero-initialized on device).
  4. Out-projection partial (this core's 512 merged dims): out_part [S, E];
     host sums the two per-batch partials and adds bo.
"""
import sys
import numpy as np
from contextlib import ExitStack

sys.path.insert(0, "/opt/trn_rl_repo")

import jax

# Persist compiled executables (incl. the embedded NEFF) across processes so
# repeat invocations skip the multi-minute neuronxcc compile.
try:
    jax.config.update("jax_compilation_cache_dir", "/tmp/jax_mha_cache")
    jax.config.update("jax_persistent_cache_min_compile_time_secs", 0.0)
    jax.config.update("jax_persistent_cache_min_entry_size_bytes", 0)
except Exception:
    pass

import concourse.bass as bass
import concourse.mybir as mybir
import concourse.tile as tile
from concourse import bacc
from concourse.bass_utils import run_bass_kernel_spmd

F32 = mybir.dt.float32
F32R = mybir.dt.float32r
BF16 = mybir.dt.bfloat16
import os as _os
MMDT = BF16 if _os.environ.get("MHA_MMDT", "bf16") == "bf16" else F32R
AF = mybir.ActivationFunctionType
OP = mybir.AluOpType

B, S, E, H, Dh = 4, 2048, 1024, 16, 64
HPC = H // 2          # 8 heads per core
JW = HPC * Dh         # 512 local merged width
NI = S // 128         # 16 i-tiles (128-wide)
NI5 = S // 512        # 4 i-tiles (512-wide)
NJ = S // 128         # 16 j-tiles (128-wide, T-phase)
NJ5 = S // 512        # 4 j-tiles (512-wide, U-phase)
NE = E // 128         # 8 e-tiles
SCALE = 1.0 / np.sqrt(Dh)
C_OFF = 12.0          # exp offset; softmax is invariant to it

_programs = {}


def build_program(causal: bool):
    nc = bacc.Bacc("TRN2", target_bir_lowering=False, debug=False, num_devices=8)

    # ---- DRAM I/O ----
    xqT_d = nc.dram_tensor("xqT", [E, S], MMDT, kind="ExternalInput").ap()
    xkT_d = nc.dram_tensor("xkT", [E, S], MMDT, kind="ExternalInput").ap()
    xvT_d = nc.dram_tensor("xvT", [E, S], MMDT, kind="ExternalInput").ap()
    wqT_d = nc.dram_tensor("wqT", [E, JW], MMDT, kind="ExternalInput").ap()
    wkT_d = nc.dram_tensor("wkT", [E, JW], MMDT, kind="ExternalInput").ap()
    wvT_d = nc.dram_tensor("wvT", [E, JW], MMDT, kind="ExternalInput").ap()
    bqp_d = nc.dram_tensor("bqp", [128, 4], F32, kind="ExternalInput").ap()
    bkp_d = nc.dram_tensor("bkp", [128, 4], F32, kind="ExternalInput").ap()
    bvr_d = nc.dram_tensor("bvr", [1, JW], MMDT, kind="ExternalInput").ap()
    woT_d = nc.dram_tensor("woT", [JW, E], MMDT, kind="ExternalInput").ap()
    mU_d = nc.dram_tensor("maskU", [128, 4, 512], F32, kind="ExternalInput").ap()
    mT_d = nc.dram_tensor("maskT", [128, 4, 512], F32, kind="ExternalInput").ap()

    attn_d = nc.dram_tensor("attn", [HPC, S, S], F32, kind="ExternalOutput").ap()
    outp_d = nc.dram_tensor("outp", [S, E], F32, kind="ExternalOutput").ap()

    with tile.TileContext(nc) as tc, ExitStack() as ctx:
        perm = ctx.enter_context(tc.tile_pool(name="perm", bufs=1))

        # persistent tensors
        qhT = perm.tile([128, 4, S], MMDT)      # partitions: local head dims
        khT = perm.tile([128, 4, S], MMDT)
        vh = perm.tile([128, NJ, HPC * (Dh + 1)], MMDT)  # [s-chunk, 8*(64+1)]
        merged = perm.tile([128, 4, S], MMDT)   # normalized (A@V).T
        if causal:
            mU = perm.tile([128, 4, 512], F32)
            mT = perm.tile([128, 4, 512], F32)
            nc.sync.dma_start(mU[:], mU_d[:])
            nc.sync.dma_start(mT[:], mT_d[:])
        bqp = perm.tile([128, 4], F32)
        bkp = perm.tile([128, 4], F32)
        nc.sync.dma_start(bqp[:], bqp_d[:])
        nc.sync.dma_start(bkp[:], bkp_d[:])
        bvr = perm.tile([1, JW], MMDT)
        nc.sync.dma_start(bvr[:], bvr_d[:])
        ones_r = perm.tile([1, 128], MMDT)
        if MMDT == F32R:
            nc.vector.memset(ones_r[:].bitcast(F32), 1.0)
        else:
            nc.vector.memset(ones_r[:], 1.0)
        ones1 = perm.tile([1, 1], F32)
        nc.vector.memset(ones1[:], 1.0)
        biasc = perm.tile([128, 1], F32)
        nc.vector.memset(biasc[:], -C_OFF)
        # ones columns of vh (col 64 of each head's 65-wide group)
        vh_g = vh[:].rearrange("p nj (h d) -> p nj h d", d=Dh + 1)
        if MMDT == F32R:
            nc.vector.memset(vh_g[:, :, :, Dh:Dh + 1].bitcast(F32), 1.0)
        else:
            nc.vector.memset(vh_g[:, :, :, Dh:Dh + 1], 1.0)

        # ---------------- projections ----------------
        with tc.tile_pool(name="wproj", bufs=1) as wpool, \
             tc.tile_pool(name="xstage", bufs=3) as xpool, \
             tc.tile_pool(name="pproj", bufs=1, space="PSUM") as ppool:
            wq = wpool.tile([128, NE, JW], MMDT)
            wk = wpool.tile([128, NE, JW], MMDT)
            wv = wpool.tile([128, NE, JW], MMDT)
            nc.sync.dma_start(wq[:], wqT_d.rearrange("(ne p) j -> p ne j", p=128))
            nc.sync.dma_start(wk[:], wkT_d.rearrange("(ne p) j -> p ne j", p=128))
            nc.sync.dma_start(wv[:], wvT_d.rearrange("(ne p) j -> p ne j", p=128))

            for st in range(4):            # s-tiles of 512
                s0 = st * 512
                # --- q and k: out [j, s] ---
                for which, xd, w, dst, bias in (
                    ("q", xqT_d, wq, qhT, bqp), ("k", xkT_d, wk, khT, bkp),
                ):
                    pss = [ppool.tile([128, 512], F32, tag=f"pqk{j}", name=f"pqk{j}")
                           for j in range(4)]
                    for e in range(NE):
                        xs = xpool.tile([128, 512], MMDT, tag="xs")
                        nc.sync.dma_start(xs[:], xd[e * 128:(e + 1) * 128, s0:s0 + 512])
                        for j in range(4):
                            nc.tensor.matmul(
                                pss[j][:], w[:, e, j * 128:(j + 1) * 128], xs[:],
                                start=(e == 0), stop=(e == NE - 1))
                    for j in range(4):
                        nc.vector.tensor_scalar_add(dst[:, j, s0:s0 + 512], pss[j][:],
                                                    bias[:, j:j + 1])
                # --- v: out [s, d] with ones-row bias augmentation ---
                pss = [ppool.tile([128, JW], F32, tag=f"pv{sc}", name=f"pv{sc}") for sc in range(4)]
                for e in range(NE):
                    xs = xpool.tile([128, 512], MMDT, tag="xs")
                    nc.sync.dma_start(xs[:], xvT_d[e * 128:(e + 1) * 128, s0:s0 + 512])
                    for sc in range(4):
                        nc.tensor.matmul(
                            pss[sc][:], xs[:, sc * 128:(sc + 1) * 128], wv[:, e, :],
                            start=(e == 0), stop=False)
                for sc in range(4):
                    nc.tensor.matmul(pss[sc][:], ones_r[:], bvr[:],
                                     start=False, stop=True)
                    gsc = st * 4 + sc
                    nc.vector.tensor_copy(
                        vh[:, gsc, :].rearrange("p (h d) -> p h d", d=Dh + 1)[:, :, 0:Dh],
                        pss[sc][:].rearrange("p (h d) -> p h d", d=Dh))

        # ---------------- attention ----------------
        with tc.tile_pool(name="pst", bufs=2, space="PSUM") as pst, \
             tc.tile_pool(name="pav", bufs=2, space="PSUM") as pav, \
             tc.tile_pool(name="prb", bufs=1, space="PSUM") as prb, \
             tc.tile_pool(name="ppb", bufs=1, space="PSUM") as ppb, \
             tc.tile_pool(name="psu", bufs=2, space="PSUM") as psu, \
             tc.tile_pool(name="expt", bufs=4) as expt_pool, \
             tc.tile_pool(name="rows", bufs=2) as rows_pool, \
             tc.tile_pool(name="bias", bufs=2) as bias_pool, \
             tc.tile_pool(name="arow", bufs=3) as arow_pool:
            for h in range(HPC):
                base = (h % 2) * 64
                jt = h // 2
                bias_all = bias_pool.tile([128, NI], F32, tag="bias_all")
                sums_all = rows_pool.tile([1, S], F32, tag="sums_all")
                # ---- T phase: scores.T, exp, A@V, denominators ----
                for ti5 in range(NI5):
                    i0 = ti5 * 512
                    ntj = 4 * (ti5 + 1) if causal else NJ
                    av = pav.tile([Dh + 1, 512], F32, tag="av")
                    for tj in range(ntj):
                        ps = pst.tile([128, 512], F32, tag="pst")
                        nc.tensor.matmul(ps[:], khT[base:base + 64, jt, tj * 128:(tj + 1) * 128],
                                         qhT[base:base + 64, jt, i0:i0 + 512],
                                         start=True, stop=True)
                        if causal and tj >= 4 * ti5:
                            nc.vector.tensor_tensor(out=ps[:], in0=ps[:],
                                                    in1=mT[:, tj - 4 * ti5, :], op=OP.add)
                        ex = expt_pool.tile([128, 512], MMDT, tag="expt")
                        nc.scalar.activation(ex[:], ps[:], AF.Exp,
                                             bias=biasc[:], scale=float(SCALE))
                        nc.tensor.matmul(av[:], vh[:, tj, h * (Dh + 1):(h + 1) * (Dh + 1)],
                                         ex[:], start=(tj == 0), stop=(tj == ntj - 1))
                    # stash denominators (av row 64) and unnormalized A@V
                    nc.vector.tensor_copy(sums_all[:, i0:i0 + 512], av[Dh:Dh + 1, :])
                    nc.vector.tensor_copy(merged[base:base + 64, jt, i0:i0 + 512],
                                          av[0:Dh, :])
                # ---- per-head plumbing (batched: 1 Ln + 1 Exp table swap) ----
                ln_r = rows_pool.tile([1, S], F32, tag="ln")
                nc.scalar.activation(ln_r[:], sums_all[:], AF.Ln, bias=0.0, scale=1.0)
                r_all = rows_pool.tile([1, S], MMDT, tag="rr")
                nc.scalar.activation(r_all[:], ln_r[:], AF.Exp, bias=0.0, scale=-1.0)
                ngc = rows_pool.tile([1, S], F32, tag="ngc")
                nc.vector.tensor_scalar(out=ngc[:], in0=ln_r[:], scalar1=-1.0,
                                        scalar2=-C_OFF, op0=OP.mult, op1=OP.add)
                for ti5 in range(NI5):
                    i0 = ti5 * 512
                    pb = ppb.tile([128, 4], F32, tag="pb")
                    for k in range(4):
                        nc.tensor.transpose(pb[:, k:k + 1],
                                            ngc[:, i0 + k * 128:i0 + (k + 1) * 128],
                                            ones1[:])
                    nc.vector.tensor_copy(bias_all[:, ti5 * 4:(ti5 + 1) * 4], pb[:])
                    # broadcast 1/sums and normalize A@V in place
                    rb = prb.tile([Dh, 512], F32, tag="rb")
                    nc.tensor.matmul(rb[:], ones_r[:, 0:Dh], r_all[:, i0:i0 + 512],
                                     start=True, stop=True)
                    nc.vector.tensor_tensor(
                        out=merged[base:base + 64, jt, i0:i0 + 512],
                        in0=merged[base:base + 64, jt, i0:i0 + 512],
                        in1=rb[:], op=OP.mult)
                # ---- U phase: normalized attention weights out ----
                for ti in range(NI):
                    ntj5 = (ti // 4) + 1 if causal else NJ5
                    ar = arow_pool.tile([128, S], F32, tag="arow")
                    for tj5 in range(ntj5):
                        j0 = tj5 * 512
                        ps = psu.tile([128, 512], F32, tag="psu")
                        nc.tensor.matmul(ps[:], qhT[base:base + 64, jt, ti * 128:(ti + 1) * 128],
                                         khT[base:base + 64, jt, j0:j0 + 512],
                                         start=True, stop=True)
                        if causal and tj5 == ntj5 - 1:
                            nc.vector.tensor_tensor(out=ps[:], in0=ps[:],
                                                    in1=mU[:, ti % 4, :], op=OP.add)
                        nc.scalar.activation(ar[:, j0:j0 + 512], ps[:], AF.Exp,
                                             bias=bias_all[:, ti:ti + 1], scale=float(SCALE))
                    nc.sync.dma_start(attn_d[h, ti * 128:(ti + 1) * 128, 0:ntj5 * 512],
                                      ar[:, 0:ntj5 * 512])

        # ---------------- out projection (partial) ----------------
        with tc.tile_pool(name="wo", bufs=1) as wopool, \
             tc.tile_pool(name="po", bufs=3, space="PSUM") as popool, \
             tc.tile_pool(name="oev", bufs=3) as oevpool:
            wo = wopool.tile([128, 4, E], MMDT)
            nc.sync.dma_start(wo[:], woT_d.rearrange("(dt p) e -> p dt e", p=128))
            for sc in range(NJ):           # 16 s-chunks of 128
                for et in range(2):        # e-tiles of 512
                    ps = popool.tile([128, 512], F32, tag="po")
                    for dt in range(4):
                        nc.tensor.matmul(ps[:], merged[:, dt, sc * 128:(sc + 1) * 128],
                                         wo[:, dt, et * 512:(et + 1) * 512],
                                         start=(dt == 0), stop=(dt == 3))
                    ot = oevpool.tile([128, 512], F32, tag="oev")
                    nc.vector.tensor_copy(ot[:], ps[:])
                    nc.sync.dma_start(
                        outp_d[sc * 128:(sc + 1) * 128, et * 512:(et + 1) * 512], ot[:])

    nc.compile()
    return nc


def _get_program(causal: bool):
    if causal not in _programs:
        _programs[causal] = build_program(causal)
    return _programs[causal]


def _host_masks():
    r = np.arange(128)[:, None]
    c = np.arange(512)[None, :]
    mU = np.zeros((4, 128, 512), dtype=np.float32)
    mT = np.zeros((4, 128, 512), dtype=np.float32)
    for p in range(4):
        mU[p] = np.where(c <= 128 * p + r, 0.0, -1e9)
        mT[p] = np.where(c >= r + 128 * p, 0.0, -1e9)
    # device layout: [128 partitions, 4 patterns, 512]
    return (np.ascontiguousarray(mU.transpose(1, 0, 2)),
            np.ascontiguousarray(mT.transpose(1, 0, 2)))


def _numpy_fallback(q, k, v, mask, Wq, bq, Wk, bk, Wv, bv, Wo, bo):
    def split_heads(x):
        return x.reshape(B, S, H, Dh).transpose(0, 2, 1, 3)
    qh = split_heads(q @ Wq.T + bq)
    kh = split_heads(k @ Wk.T + bk)
    vh = split_heads(v @ Wv.T + bv)
    scores = np.einsum("bhqd,bhkd->bhqk", qh, kh) * np.float32(SCALE)
    scores = np.where(np.asarray(mask) == 0, np.float32(-1e9), scores)
    m = scores.max(axis=-1, keepdims=True)
    e = np.exp(scores - m)
    aw = e / e.sum(axis=-1, keepdims=True)
    attn_out = np.einsum("bhqk,bhkd->bhqd", aw, vh)
    mg = attn_out.transpose(0, 2, 1, 3).reshape(B, S, E)
    return (mg @ Wo.T + bo).astype(np.float32), aw.astype(np.float32)


def kernel(q, k, v, mask, Wq, bq, Wk, bk, Wv, bv, Wo, bo):
    q = np.asarray(q, dtype=np.float32)
    k = np.asarray(k, dtype=np.float32)
    v = np.asarray(v, dtype=np.float32)
    mask = np.asarray(mask)
    Wq, bq = np.asarray(Wq, np.float32), np.asarray(bq, np.float32)
    Wk, bk = np.asarray(Wk, np.float32), np.asarray(bk, np.float32)
    Wv, bv = np.asarray(Wv, np.float32), np.asarray(bv, np.float32)
    Wo, bo = np.asarray(Wo, np.float32), np.asarray(bo, np.float32)

    m2 = np.broadcast_to(mask, (1, 1, S, S)).reshape(S, S)
    tril = np.tril(np.ones((S, S), dtype=m2.dtype))
    if np.array_equal(m2, tril):
        causal = True
    elif np.all(m2 != 0):
        causal = False
    else:
        return _numpy_fallback(q, k, v, mask, Wq, bq, Wk, bk, Wv, bv, Wo, bo)

    nc = _get_program(causal)
    mU, mT = _host_masks()

    if MMDT == BF16:
        import ml_dtypes
        mmnp = ml_dtypes.bfloat16
    else:
        mmnp = np.float32

    def mc(a):
        return np.ascontiguousarray(a).astype(mmnp)

    in_maps = []
    for c in range(8):
        b, hh = c // 2, c % 2
        sl = slice(hh * JW, (hh + 1) * JW)
        in_maps.append({
            "xqT": mc(q[b].T),
            "xkT": mc(k[b].T),
            "xvT": mc(v[b].T),
            "wqT": mc(Wq[sl, :].T),
            "wkT": mc(Wk[sl, :].T),
            "wvT": mc(Wv[sl, :].T),
            "bqp": np.ascontiguousarray(bq[sl].reshape(4, 128).T),
            "bkp": np.ascontiguousarray(bk[sl].reshape(4, 128).T),
            "bvr": mc(bv[sl].reshape(1, JW)),
            "woT": mc(Wo[:, sl].T),
            "maskU": mU, "maskT": mT,
        })

    res = run_bass_kernel_spmd(nc, in_maps, list(range(8)))

    aw = np.empty((B, H, S, S), dtype=np.float32)
    out = np.empty((B, S, E), dtype=np.float32)
    for b in range(B):
        aw[b, 0:HPC] = res.results[2 * b]["attn"]
        aw[b, HPC:H] = res.results[2 * b + 1]["attn"]
        out[b] = res.results[2 * b]["outp"] + res.results[2 * b + 1]["outp"] + bo
    return out, aw


# revision 21
# speedup vs baseline: 1.4084x; 1.0552x over previous
"""Multi-head attention (B=4,S=2048,E=1024,H=16,Dh=64) on 8 TRN2 NeuronCores.

Sharding: core c handles batch b=c//2 and heads hh*8..hh*8+8 (hh=c%2).
Returns (out [B,S,E], attn_weights [B,H,S,S]) matching the reference.

Per-core device program (all matmuls float32r, fp32 I/O):
  1. Projections: qhT/khT [d_local, S] (head-dim on partitions), vh [S, d_local]
     natural layout augmented with a ones column per head (row-sum trick).
  2. T-phase per (head, i-tile of 512): scores.T blocks [128 j, 512 i] -> exp
     (scale=1/8, bias=-C) -> A@V accumulation; vh's ones column makes PSUM
     row 64 the softmax denominators. Denominators -> ln -> per-partition
     bias tiles (PE row-transpose); reciprocal row -> PE-broadcast -> A@V
     normalization at evacuation.
  3. U-phase per (head, i-tile of 128): scores blocks [128 i, 512 j] -> exp
     with bias = -C - ln(sum_i) => normalized attention weights directly;
     causal-masked entries are exact zeros; upper triangle blocks are never
     written (outputs are zero-initialized on device).
  4. Out-projection partial (this core's 512 merged dims): out_part [S, E];
     host sums the two per-batch partials and adds bo.
"""
import sys
import numpy as np
from contextlib import ExitStack

sys.path.insert(0, "/opt/trn_rl_repo")

import jax

# Persist compiled executables (incl. the embedded NEFF) across processes so
# repeat invocations skip the multi-minute neuronxcc compile.
try:
    jax.config.update("jax_compilation_cache_dir", "/tmp/jax_mha_cache")
    jax.config.update("jax_persistent_cache_min_compile_time_secs", 0.0)
    jax.config.update("jax_persistent_cache_min_entry_size_bytes", 0)
except Exception:
    pass

import concourse.bass as bass
import concourse.mybir as mybir
import concourse.tile as tile
from concourse import bacc
from concourse.bass_utils import run_bass_kernel_spmd

F32 = mybir.dt.float32
F32R = mybir.dt.float32r
BF16 = mybir.dt.bfloat16
import os as _os
MMDT = BF16 if _os.environ.get("MHA_MMDT", "bf16") == "bf16" else F32R
AF = mybir.ActivationFunctionType
OP = mybir.AluOpType

B, S, E, H, Dh = 4, 2048, 1024, 16, 64
HPC = H // 2          # 8 heads per core
JW = HPC * Dh         # 512 local merged width
NI = S // 128         # 16 i-tiles (128-wide)
NI5 = S // 512        # 4 i-tiles (512-wide)
NJ = S // 128         # 16 j-tiles (128-wide, T-phase)
NJ5 = S // 512        # 4 j-tiles (512-wide, U-phase)
NE = E // 128         # 8 e-tiles
SCALE = 1.0 / np.sqrt(Dh)
C_OFF = 12.0          # exp offset; softmax is invariant to it

_programs = {}


def build_program(causal: bool):
    nc = bacc.Bacc("TRN2", target_bir_lowering=False, debug=False, num_devices=8)

    # ---- DRAM I/O ----
    xqT_d = nc.dram_tensor("xqT", [E, S], MMDT, kind="ExternalInput").ap()
    xkT_d = nc.dram_tensor("xkT", [E, S], MMDT, kind="ExternalInput").ap()
    xvT_d = nc.dram_tensor("xvT", [E, S], MMDT, kind="ExternalInput").ap()
    wqT_d = nc.dram_tensor("wqT", [E, JW], MMDT, kind="ExternalInput").ap()
    wkT_d = nc.dram_tensor("wkT", [E, JW], MMDT, kind="ExternalInput").ap()
    wvT_d = nc.dram_tensor("wvT", [E, JW], MMDT, kind="ExternalInput").ap()
    bqp_d = nc.dram_tensor("bqp", [128, 4], F32, kind="ExternalInput").ap()
    bkp_d = nc.dram_tensor("bkp", [128, 4], F32, kind="ExternalInput").ap()
    bvr_d = nc.dram_tensor("bvr", [1, JW], MMDT, kind="ExternalInput").ap()
    woT_d = nc.dram_tensor("woT", [JW, E], MMDT, kind="ExternalInput").ap()
    mU_d = nc.dram_tensor("maskU", [128, 4, 512], F32, kind="ExternalInput").ap()
    mT_d = nc.dram_tensor("maskT", [128, 4, 512], F32, kind="ExternalInput").ap()

    attn_d = nc.dram_tensor("attn", [HPC, S, S], F32, kind="ExternalOutput").ap()
    outp_d = nc.dram_tensor("outp", [S, E], F32, kind="ExternalOutput").ap()

    with tile.TileContext(nc) as tc, ExitStack() as ctx:
        perm = ctx.enter_context(tc.tile_pool(name="perm", bufs=1))

        # persistent tensors
        qhT = perm.tile([128, 4, S], MMDT)      # partitions: local head dims
        khT = perm.tile([128, 4, S], MMDT)
        vh = perm.tile([128, NJ, HPC * (Dh + 1)], MMDT)  # [s-chunk, 8*(64+1)]
        merged = perm.tile([128, 4, S], MMDT)   # normalized (A@V).T
        if causal:
            mU = perm.tile([128, 4, 512], F32)
            mT = perm.tile([128, 4, 512], F32)
            nc.sync.dma_start(mU[:], mU_d[:])
            nc.sync.dma_start(mT[:], mT_d[:])
        bqp = perm.tile([128, 4], F32)
        bkp = perm.tile([128, 4], F32)
        nc.sync.dma_start(bqp[:], bqp_d[:])
        nc.sync.dma_start(bkp[:], bkp_d[:])
        bvr = perm.tile([1, JW], MMDT)
        nc.sync.dma_start(bvr[:], bvr_d[:])
        ones_r = perm.tile([1, 128], MMDT)
        if MMDT == F32R:
            nc.vector.memset(ones_r[:].bitcast(F32), 1.0)
        else:
            nc.vector.memset(ones_r[:], 1.0)
        ones1 = perm.tile([1, 1], F32)
        nc.vector.memset(ones1[:], 1.0)
        biasc = perm.tile([128, 1], F32)
        nc.vector.memset(biasc[:], -C_OFF)
        # ones columns of vh (col 64 of each head's 65-wide group)
        vh_g = vh[:].rearrange("p nj (h d) -> p nj h d", d=Dh + 1)
        if MMDT == F32R:
            nc.vector.memset(vh_g[:, :, :, Dh:Dh + 1].bitcast(F32), 1.0)
        else:
            nc.vector.memset(vh_g[:, :, :, Dh:Dh + 1], 1.0)

        # ---------------- projections ----------------
        with tc.tile_pool(name="wproj", bufs=1) as wpool, \
             tc.tile_pool(name="xstage", bufs=3) as xpool, \
             tc.tile_pool(name="pproj", bufs=1, space="PSUM") as ppool:
            wq = wpool.tile([128, NE, JW], MMDT)
            wk = wpool.tile([128, NE, JW], MMDT)
            wv = wpool.tile([128, NE, JW], MMDT)
            nc.sync.dma_start(wq[:], wqT_d.rearrange("(ne p) j -> p ne j", p=128))
            nc.sync.dma_start(wk[:], wkT_d.rearrange("(ne p) j -> p ne j", p=128))
            nc.sync.dma_start(wv[:], wvT_d.rearrange("(ne p) j -> p ne j", p=128))

            for st in range(4):            # s-tiles of 512
                s0 = st * 512
                # --- q and k: out [j, s] ---
                for which, xd, w, dst, bias in (
                    ("q", xqT_d, wq, qhT, bqp), ("k", xkT_d, wk, khT, bkp),
                ):
                    pss = [ppool.tile([128, 512], F32, tag=f"pqk{j}", name=f"pqk{j}")
                           for j in range(4)]
                    for e in range(NE):
                        xs = xpool.tile([128, 512], MMDT, tag="xs")
                        nc.sync.dma_start(xs[:], xd[e * 128:(e + 1) * 128, s0:s0 + 512])
                        for j in range(4):
                            nc.tensor.matmul(
                                pss[j][:], w[:, e, j * 128:(j + 1) * 128], xs[:],
                                start=(e == 0), stop=(e == NE - 1))
                    for j in range(4):
                        nc.vector.tensor_scalar_add(dst[:, j, s0:s0 + 512], pss[j][:],
                                                    bias[:, j:j + 1])
                # --- v: out [s, d] with ones-row bias augmentation ---
                pss = [ppool.tile([128, JW], F32, tag=f"pv{sc}", name=f"pv{sc}") for sc in range(4)]
                for e in range(NE):
                    xs = xpool.tile([128, 512], MMDT, tag="xs")
                    nc.sync.dma_start(xs[:], xvT_d[e * 128:(e + 1) * 128, s0:s0 + 512])
                    for sc in range(4):
                        nc.tensor.matmul(
                            pss[sc][:], xs[:, sc * 128:(sc + 1) * 128], wv[:, e, :],
                            start=(e == 0), stop=False)
                for sc in range(4):
                    nc.tensor.matmul(pss[sc][:], ones_r[:], bvr[:],
                                     start=False, stop=True)
                    gsc = st * 4 + sc
                    nc.vector.tensor_copy(
                        vh[:, gsc, :].rearrange("p (h d) -> p h d", d=Dh + 1)[:, :, 0:Dh],
                        pss[sc][:].rearrange("p (h d) -> p h d", d=Dh))

        # ---------------- attention ----------------
        with tc.tile_pool(name="pst", bufs=2, space="PSUM") as pst, \
             tc.tile_pool(name="pav", bufs=2, space="PSUM") as pav, \
             tc.tile_pool(name="prb", bufs=1, space="PSUM") as prb, \
             tc.tile_pool(name="ppb", bufs=1, space="PSUM") as ppb, \
             tc.tile_pool(name="psu", bufs=2, space="PSUM") as psu, \
             tc.tile_pool(name="expt", bufs=4) as expt_pool, \
             tc.tile_pool(name="rows", bufs=2) as rows_pool, \
             tc.tile_pool(name="bias", bufs=2) as bias_pool, \
             tc.tile_pool(name="arow", bufs=3) as arow_pool:
            hstate = {}

            def emit_T(h):
                base = (h % 2) * 64
                jt = h // 2
                bias_all = bias_pool.tile([128, NI], F32, tag="bias_all",
                                          name=f"bias_all{h}")
                sums_all = rows_pool.tile([1, S], F32, tag="sums_all",
                                          name=f"sums_all{h}")
                hstate[h] = (bias_all, sums_all)
                for ti5 in range(NI5):
                    i0 = ti5 * 512
                    ntj = 4 * (ti5 + 1) if causal else NJ
                    av = pav.tile([Dh + 1, 512], F32, tag="av", name=f"av{h}_{ti5}")
                    for tj in range(ntj):
                        ps = pst.tile([128, 512], F32, tag="pst", name=f"pst{h}_{ti5}_{tj}")
                        nc.tensor.matmul(ps[:], khT[base:base + 64, jt, tj * 128:(tj + 1) * 128],
                                         qhT[base:base + 64, jt, i0:i0 + 512],
                                         start=True, stop=True)
                        if causal and tj >= 4 * ti5:
                            nc.vector.tensor_tensor(out=ps[:], in0=ps[:],
                                                    in1=mT[:, tj - 4 * ti5, :], op=OP.add)
                        ex = expt_pool.tile([128, 512], MMDT, tag="expt",
                                            name=f"ex{h}_{ti5}_{tj}")
                        nc.scalar.activation(ex[:], ps[:], AF.Exp,
                                             bias=biasc[:], scale=float(SCALE))
                        nc.tensor.matmul(av[:], vh[:, tj, h * (Dh + 1):(h + 1) * (Dh + 1)],
                                         ex[:], start=(tj == 0), stop=(tj == ntj - 1))
                    # stash denominators (av row 64) and unnormalized A@V
                    nc.vector.tensor_copy(sums_all[:, i0:i0 + 512], av[Dh:Dh + 1, :])
                    nc.vector.tensor_copy(merged[base:base + 64, jt, i0:i0 + 512],
                                          av[0:Dh, :])

            def emit_plumb(h):
                base = (h % 2) * 64
                jt = h // 2
                bias_all, sums_all = hstate[h]
                ln_r = rows_pool.tile([1, S], F32, tag="ln", name=f"ln{h}")
                nc.scalar.activation(ln_r[:], sums_all[:], AF.Ln, bias=0.0, scale=1.0)
                r_all = rows_pool.tile([1, S], MMDT, tag="rr", name=f"rr{h}")
                nc.scalar.activation(r_all[:], ln_r[:], AF.Exp, bias=0.0, scale=-1.0)
                ngc = rows_pool.tile([1, S], F32, tag="ngc", name=f"ngc{h}")
                nc.vector.tensor_scalar(out=ngc[:], in0=ln_r[:], scalar1=-1.0,
                                        scalar2=-C_OFF, op0=OP.mult, op1=OP.add)
                for ti5 in range(NI5):
                    i0 = ti5 * 512
                    pb = ppb.tile([128, 4], F32, tag="pb", name=f"pb{h}_{ti5}")
                    for k in range(4):
                        nc.tensor.transpose(pb[:, k:k + 1],
                                            ngc[:, i0 + k * 128:i0 + (k + 1) * 128],
                                            ones1[:])
                    nc.vector.tensor_copy(bias_all[:, ti5 * 4:(ti5 + 1) * 4], pb[:])
                    rb = prb.tile([Dh, 512], F32, tag="rb", name=f"rb{h}_{ti5}")
                    nc.tensor.matmul(rb[:], ones_r[:, 0:Dh], r_all[:, i0:i0 + 512],
                                     start=True, stop=True)
                    nc.vector.tensor_tensor(
                        out=merged[base:base + 64, jt, i0:i0 + 512],
                        in0=merged[base:base + 64, jt, i0:i0 + 512],
                        in1=rb[:], op=OP.mult)

            def emit_U(h):
                base = (h % 2) * 64
                jt = h // 2
                bias_all = hstate[h][0]
                for ti in range(NI):
                    ntj5 = (ti // 4) + 1 if causal else NJ5
                    ar = arow_pool.tile([128, S], F32, tag="arow", name=f"ar{h}_{ti}")
                    for tj5 in range(ntj5):
                        j0 = tj5 * 512
                        ps = psu.tile([128, 512], F32, tag="psu", name=f"psu{h}_{ti}_{tj5}")
                        nc.tensor.matmul(ps[:], qhT[base:base + 64, jt, ti * 128:(ti + 1) * 128],
                                         khT[base:base + 64, jt, j0:j0 + 512],
                                         start=True, stop=True)
                        if causal and tj5 == ntj5 - 1:
                            nc.vector.tensor_tensor(out=ps[:], in0=ps[:],
                                                    in1=mU[:, ti % 4, :], op=OP.add)
                        nc.scalar.activation(ar[:, j0:j0 + 512], ps[:], AF.Exp,
                                             bias=bias_all[:, ti:ti + 1], scale=float(SCALE))
                    nc.sync.dma_start(attn_d[h, ti * 128:(ti + 1) * 128, 0:ntj5 * 512],
                                      ar[:, 0:ntj5 * 512])

            # software-pipeline by one head: T(h) is emitted before
            # plumb/U of head h-1 so PE always has ready matmuls while the
            # serial Ln/Exp plumbing chain of the previous head runs on ACT.
            for h in range(HPC):
                emit_T(h)
                if h >= 1:
                    emit_plumb(h - 1)
                    emit_U(h - 1)
            emit_plumb(HPC - 1)
            emit_U(HPC - 1)

        # ---------------- out projection (partial) ----------------
        with tc.tile_pool(name="wo", bufs=1) as wopool, \
             tc.tile_pool(name="po", bufs=3, space="PSUM") as popool, \
             tc.tile_pool(name="oev", bufs=3) as oevpool:
            wo = wopool.tile([128, 4, E], MMDT)
            nc.sync.dma_start(wo[:], woT_d.rearrange("(dt p) e -> p dt e", p=128))
            for sc in range(NJ):           # 16 s-chunks of 128
                for et in range(2):        # e-tiles of 512
                    ps = popool.tile([128, 512], F32, tag="po")
                    for dt in range(4):
                        nc.tensor.matmul(ps[:], merged[:, dt, sc * 128:(sc + 1) * 128],
                                         wo[:, dt, et * 512:(et + 1) * 512],
                                         start=(dt == 0), stop=(dt == 3))
                    ot = oevpool.tile([128, 512], F32, tag="oev")
                    nc.vector.tensor_copy(ot[:], ps[:])
                    nc.sync.dma_start(
                        outp_d[sc * 128:(sc + 1) * 128, et * 512:(et + 1) * 512], ot[:])

    nc.compile()
    return nc


def _get_program(causal: bool):
    if causal not in _programs:
        _programs[causal] = build_program(causal)
    return _programs[causal]


def _host_masks():
    r = np.arange(128)[:, None]
    c = np.arange(512)[None, :]
    mU = np.zeros((4, 128, 512), dtype=np.float32)
    mT = np.zeros((4, 128, 512), dtype=np.float32)
    for p in range(4):
        mU[p] = np.where(c <= 128 * p + r, 0.0, -1e9)
        mT[p] = np.where(c >= r + 128 * p, 0.0, -1e9)
    # device layout: [128 partitions, 4 patterns, 512]
    return (np.ascontiguousarray(mU.transpose(1, 0, 2)),
            np.ascontiguousarray(mT.transpose(1, 0, 2)))


def _numpy_fallback(q, k, v, mask, Wq, bq, Wk, bk, Wv, bv, Wo, bo):
    def split_heads(x):
        return x.reshape(B, S, H, Dh).transpose(0, 2, 1, 3)
    qh = split_heads(q @ Wq.T + bq)
    kh = split_heads(k @ Wk.T + bk)
    vh = split_heads(v @ Wv.T + bv)
    scores = np.einsum("bhqd,bhkd->bhqk", qh, kh) * np.float32(SCALE)
    scores = np.where(np.asarray(mask) == 0, np.float32(-1e9), scores)
    m = scores.max(axis=-1, keepdims=True)
    e = np.exp(scores - m)
    aw = e / e.sum(axis=-1, keepdims=True)
    attn_out = np.einsum("bhqk,bhkd->bhqd", aw, vh)
    mg = attn_out.transpose(0, 2, 1, 3).reshape(B, S, E)
    return (mg @ Wo.T + bo).astype(np.float32), aw.astype(np.float32)


def kernel(q, k, v, mask, Wq, bq, Wk, bk, Wv, bv, Wo, bo):
    q = np.asarray(q, dtype=np.float32)
    k = np.asarray(k, dtype=np.float32)
    v = np.asarray(v, dtype=np.float32)
    mask = np.asarray(mask)
    Wq, bq = np.asarray(Wq, np.float32), np.asarray(bq, np.float32)
    Wk, bk = np.asarray(Wk, np.float32), np.asarray(bk, np.float32)
    Wv, bv = np.asarray(Wv, np.float32), np.asarray(bv, np.float32)
    Wo, bo = np.asarray(Wo, np.float32), np.asarray(bo, np.float32)

    m2 = np.broadcast_to(mask, (1, 1, S, S)).reshape(S, S)
    tril = np.tril(np.ones((S, S), dtype=m2.dtype))
    if np.array_equal(m2, tril):
        causal = True
    elif np.all(m2 != 0):
        causal = False
    else:
        return _numpy_fallback(q, k, v, mask, Wq, bq, Wk, bk, Wv, bv, Wo, bo)

    nc = _get_program(causal)
    mU, mT = _host_masks()

    if MMDT == BF16:
        import ml_dtypes
        mmnp = ml_dtypes.bfloat16
    else:
        mmnp = np.float32

    def mc(a):
        return np.ascontiguousarray(a).astype(mmnp)

    in_maps = []
    for c in range(8):
        b, hh = c // 2, c % 2
        sl = slice(hh * JW, (hh + 1) * JW)
        in_maps.append({
            "xqT": mc(q[b].T),
            "xkT": mc(k[b].T),
            "xvT": mc(v[b].T),
            "wqT": mc(Wq[sl, :].T),
            "wkT": mc(Wk[sl, :].T),
            "wvT": mc(Wv[sl, :].T),
            "bqp": np.ascontiguousarray(bq[sl].reshape(4, 128).T),
            "bkp": np.ascontiguousarray(bk[sl].reshape(4, 128).T),
            "bvr": mc(bv[sl].reshape(1, JW)),
            "woT": mc(Wo[:, sl].T),
            "maskU": mU, "maskT": mT,
        })

    res = run_bass_kernel_spmd(nc, in_maps, list(range(8)))

    aw = np.empty((B, H, S, S), dtype=np.float32)
    out = np.empty((B, S, E), dtype=np.float32)
    for b in range(B):
        aw[b, 0:HPC] = res.results[2 * b]["attn"]
        aw[b, HPC:H] = res.results[2 * b + 1]["attn"]
        out[b] = res.results[2 * b]["outp"] + res.results[2 * b + 1]["outp"] + bo
    return out, aw
